# revision 59
# baseline (speedup 1.0000x reference)
"""Multi-head attention (16 heads, d_model=1024, head_dim=64) on 8 trn2 cores.

Sharding: core c handles batch b = c//2 and heads [8*(c%2), 8*(c%2)+8)
(data parallel over batch x tensor parallel over heads). Each core
computes its 8 heads' Q/K/V projections, attention, and a partial output
projection; the host sums the two partial projections per batch element
(the "all-reduce") and adds the output bias.

Device-side layout is feature-major ("transposed"): projections produce
Q^T/K^T [d, t] so that the attention matmuls contract along partitions.

Inputs are pre-arranged on the HOST into the exact SBUF layouts
(x: [p, n, m, t'], w: [p, hp, m, c]) so every DMA is a full-bandwidth
contiguous transfer, and are issued in consumption order on the SP and
DVE queues only (the ACT queue stays free for the exp stream): the first
energy slot starts ~4.5us in instead of waiting ~11us for bulk input
DMAs. Unit 0 runs on a DMA-aware filler diet: Q/K n-tiles and V tiles
are pumped between energy slots in the order their x/w slabs land.

The two heads of a head-pair are processed in ONE pair-unit: their
energy matmuls are emitted back-to-back as 64x128 row tiles at
positions (0,0)/(64,0) with a dedicated top-priority band, and the
K=64 energies' exp covers both heads per key chunk. All other PE work
(attn@V chains, output projection, next pair's Q/K tiles) is flattened
into generators and pumped ~4 matmuls per exp slot so the scalar
engine streams activations back-to-back. Softmax normalization runs
off the PE entirely: row-sum reciprocals on the DVE, the partition
broadcast on the (otherwise idle) GPSIMD, and the scale on the DVE.
V bias is applied on the DVE against a broadcast tile, and the V
projection is split by head halves: heads 0-3 up front, heads 4-7
(first consumed by pair 2's attention) spread over mid-stream units.
Output staging and the out DRAM tensor are bf16 (host sums the two
partial projections per batch element in fp32).

All matmul inputs are bf16 (fp32 PSUM accumulation); softmax is
unnormalized exp (energies bounded ~|15|) with the row-sum from an
extra ones-column in the attn@V matmul.
"""

import numpy as np
import ml_dtypes

from concourse import bass, bacc, tile, mybir
from concourse.tile_rust import add_dep_helper
from concourse.bass_utils import run_bass_kernel_spmd

BF16 = ml_dtypes.bfloat16
dt = mybir.dt
AF = mybir.ActivationFunctionType

N_CORES = 8
T = 2048          # tokens per batch element
D = 1024          # model dim
FH = 512          # features (head dims) per core: 8 heads x 64
NH_LOC = 8        # heads per core
HD = 64           # head dim

_prog_cache = {}


def _build_program():
    nc = bacc.Bacc("TRN2", target_bir_lowering=False, debug=False,
                   num_devices=N_CORES)

    # Host-prearranged layouts: xn[p, n, m, t'], w[p, hp, m, c].
    xn = nc.dram_tensor("xn", [128, 4, 8, 512], dt.bfloat16,
                        kind="ExternalInput").ap()
    wqr = nc.dram_tensor("wqr", [128, 4, 8, 128], dt.bfloat16,
                         kind="ExternalInput").ap()
    wkr = nc.dram_tensor("wkr", [128, 4, 8, 128], dt.bfloat16,
                         kind="ExternalInput").ap()
    wvr = nc.dram_tensor("wvr", [128, 2, 8, 256], dt.bfloat16,
                         kind="ExternalInput").ap()
    bqT = nc.dram_tensor("bqT", [128, 4], dt.float32, kind="ExternalInput").ap()
    bkT = nc.dram_tensor("bkT", [128, 4], dt.float32, kind="ExternalInput").ap()
    bvs = nc.dram_tensor("bvs", [1, FH], dt.bfloat16, kind="ExternalInput").ap()
    wpT = nc.dram_tensor("wpT", [FH, D], dt.bfloat16, kind="ExternalInput").ap()
    ones = nc.dram_tensor("ones", [1, 128], dt.bfloat16, kind="ExternalInput").ap()
    out = nc.dram_tensor("out", [T, D], dt.bfloat16, kind="ExternalOutput").ap()

    with tile.TileContext(nc) as tc:
        _emit(tc, out, xn, wqr, wkr, wvr, bqT, bkT, bvs, wpT, ones)
    nc.compile()
    return nc


def _emit(tc, out, xn, wqr, wkr, wvr, bqT, bkT, bvs, wpT, ones):
    nc = tc.nc
    f32 = dt.float32
    bf16 = dt.bfloat16

    with (
        tc.tile_pool(name="sbp", bufs=1) as sbp,
        tc.tile_pool(name="qkv_sb", bufs=1) as qkv_sb,
        tc.tile_pool(name="pb_pool", bufs=2) as pb_pool,
        tc.tile_pool(name="rr_pool", bufs=2) as rr_pool,
        tc.tile_pool(name="bc_pool", bufs=2) as bc_pool,
        tc.tile_pool(name="ostage", bufs=3) as ostage,
        # PSUM: 4 banks for energies (2-bank tiles, ping-pong), 2 for
        # attn@V accumulators, 2 shared by V / Q,K tiles / output proj.
        tc.tile_pool(name="ps_e", bufs=2, space="PSUM") as ps_e,
        tc.tile_pool(name="ps_av", bufs=2, space="PSUM") as ps_av,
        tc.tile_pool(name="ps_misc", bufs=2, space="PSUM") as ps_misc,
    ):
        # Input DMAs in consumption order, alternating the SP ("sync") and
        # DVE ("vector") HW-DGE queues. The ACT queue carries no DMAs: its
        # sequencer must stream exps back-to-back mid-kernel.
        wk_s = sbp.tile([128, 4, 8, 128], bf16, tag="wk")
        wq_s = sbp.tile([128, 4, 8, 128], bf16, tag="wq")
        wv_s = sbp.tile([128, 2, 8, 256], bf16, tag="wv")
        x_s = sbp.tile([128, 4, 8, 512], bf16)

        # Coarse first pieces: the PE p-state ramp punishes stop-start
        # trickle feeding, so the first chain should begin only once the
        # stream can sustain it.
        nc.sync.dma_start(out=wk_s[:, 0:1], in_=wkr[:, 0:1])
        nc.scalar.dma_start(out=x_s[:, 0:1, 0:4], in_=xn[:, 0:1, 0:4])
        nc.sync.dma_start(out=x_s[:, 0:1, 4:8], in_=xn[:, 0:1, 4:8])
        nc.scalar.dma_start(out=wq_s[:, 0:1], in_=wqr[:, 0:1])
        # xn1 before the tiny bias pieces: each dma_start costs ~625ns of
        # serialized HWDGE generation, so four tiny DMAs ahead of xn1
        # would push K(0,1)'s data ~1.3us later than the pump needs it
        nc.sync.dma_start(out=x_s[:, 1:2], in_=xn[:, 1:2])
        bkT_s = sbp.tile([128, 4], f32)
        nc.sync.dma_start(out=bkT_s[:], in_=bkT)
        bqT_s = sbp.tile([128, 4], f32)
        nc.sync.dma_start(out=bqT_s[:], in_=bqT)
        ones_s = sbp.tile([1, 128], bf16)
        nc.scalar.dma_start(out=ones_s[:], in_=ones)
        bvs_s = sbp.tile([1, FH], bf16)
        nc.scalar.dma_start(out=bvs_s[:], in_=bvs)
        nc.scalar.dma_start(out=x_s[:, 2:3], in_=xn[:, 2:3])
        nc.sync.dma_start(out=wv_s[:, 0:1], in_=wvr[:, 0:1])
        nc.scalar.dma_start(out=x_s[:, 3:4], in_=xn[:, 3:4])
        nc.sync.dma_start(out=wv_s[:, 1:2], in_=wvr[:, 1:2])
        nc.scalar.dma_start(out=wk_s[:, 1:4], in_=wkr[:, 1:4])
        nc.sync.dma_start(out=wq_s[:, 1:4], in_=wqr[:, 1:4])
        wp_s = sbp.tile([128, 4, D], bf16)
        nc.scalar.dma_start(out=wp_s[:], in_=wpT.rearrange("(c p) o -> p c o", p=128))

        # QT/KT: [d-in-pair(128), head-pair(4), t]; V: [t-in-chunk(128),
        # t-chunk(16), head(8), 65] with col 64 = 1.0 (row-sum trick).
        # Q^T/K^T live only while their pair streams (+1 pair prefill):
        # 2-slot rings indexed hp % 2.
        QT_sb = qkv_sb.tile([128, 2, T], bf16)
        KT_sb = qkv_sb.tile([128, 2, T], bf16)
        V_sb = qkv_sb.tile([128, 16, NH_LOC, 65], bf16)
        nc.vector.memset(V_sb[:, :, :, 64:65], 1.0)
        bvb_s = qkv_sb.tile([128, NH_LOC, 64], bf16)
        # AttnOut^T: [f-in-chunk(128), f-chunk(4), t]
        AO_sb = qkv_sb.tile([128, 4, T], bf16)

        def emit_bvb():
            # broadcast bv across the 128 partitions via a K=1 matmul
            ps = ps_misc.tile([128, 512], f32, tag="m", name="bvb_ps")
            nc.tensor.matmul(ps[:], ones_s[0:1, 0:128], bvs_s[:],
                             start=True, stop=True)
            nc.vector.tensor_copy(bvb_s[:], ps[:].rearrange(
                "p (h d) -> p h d", h=NH_LOC))

        # ---- software-pipelined attention over 16 pair-units ----
        # All non-energy PE work (attn@V chains, projections, Q/K tiles)
        # is flattened into generators that yield after every matmul; a
        # pump dispenses ~4 of them between consecutive energy slots so
        # the activation engine is never starved for more than ~1us.
        units = [(hp, j) for hp in range(4) for j in range(4)]
        state = {}
        bg = []
        eprio = [-1000000]  # energy matmuls outrank all filler work

        def gen_qk_ntile(w_s, b_s, dst, hp, n):
            ps = ps_misc.tile([128, 512], f32, tag="m", name="qk_ps")
            for m in range(8):
                yield nc.tensor.matmul(ps[:], w_s[:, hp, m, :],
                                       x_s[:, n, m, :],
                                       start=(m == 0), stop=(m == 7))
            nc.vector.tensor_scalar_add(
                dst[:, hp % 2, n * 512:(n + 1) * 512], ps[:],
                b_s[:, hp:hp + 1])

        def gen_bvb():
            emit_bvb()
            yield None

        def gen_v_tile(t, half):
            # V (natural): out[t, d] = x[t, :].wvT[:, d]; bias on the DVE.
            # half 0 = heads 0-3 (needed by pair 0 from unit 1), half 1 =
            # heads 4-7 (pairs 2-3, not consumed until ~unit 9).
            hsl = slice(4 * half, 4 * half + 4)
            n, tq = divmod(t, 4)
            ps = ps_misc.tile([128, 512], f32, tag="m", name="v_ps")
            for m in range(8):
                yield nc.tensor.matmul(ps[:, 0:256],
                                       x_s[:, n, m, tq * 128:(tq + 1) * 128],
                                       wv_s[:, half, m, :],
                                       start=(m == 0), stop=(m == 7))
            nc.vector.tensor_add(
                V_sb[:, t, hsl, 0:64],
                ps[:, 0:256].rearrange("p (h d) -> p h d", h=4),
                bvb_s[:, hsl, :])

        def gen_av_block(u, s, lo, hi):
            # attn@V accumulation (V col 64 is ones -> row sums); the
            # closing block runs normalization part 1 (DVE): spill rows,
            # reciprocal row sums, freeing the av bank.
            hp, j = u
            st = state[u]
            if lo == 0:
                st["av"][s] = ps_av.tile([128, 512], f32, tag="av", name="av")
            av = st["av"][s]
            pb = st["pb"]
            for kc in range(lo, hi):
                yield nc.tensor.matmul(av[0:65, :],
                                       V_sb[:, kc, 2 * hp + s, 0:65],
                                       pb[:, kc, s, :],
                                       start=(kc == 0), stop=(kc == 15))
            if hi == 16:
                # reciprocal first: it heads the serial norm chain (rr ->
                # rrb -> gpsimd broadcast -> scale); the bf16 spill then
                # overlaps the broadcast instead of delaying it.
                rr = rr_pool.tile([1, 512], f32, tag="rr", bufs=2)
                nc.vector.reciprocal(rr[:], av[64:65, :])
                rrb = rr_pool.tile([1, 512], bf16, tag="rrb", bufs=4)
                nc.vector.tensor_copy(rrb[:], rr[:])
                avd = bc_pool.tile([64, 512], bf16, tag="avd", bufs=3)
                if u == (3, 3):
                    # post-last-exp the ACT engine is idle: spilling there
                    # keeps the serial DVE norm chain (rr/rrb/mul) short
                    nc.scalar.activation(avd[:], av[0:64, :], AF.Copy)
                else:
                    nc.vector.tensor_copy(avd[:], av[0:64, :])
                st["avd"][s] = avd
                st["rrb"][s] = rrb

        def gen_norm_b(u, s):
            # part 2: broadcast 1/rowsum across the 64 head-dim partitions
            # on the (otherwise idle) GPSIMD engine, then scale into
            # AttnOut^T. Keeping this off the PE and out of the ps_misc
            # rotation stops the reciprocal chain from serializing the
            # Q/K filler tiles. For the LAST unit the bf16 spill is skipped
            # (no next av user): scale straight from the psum accumulator,
            # shortening the tail's serial norm chain.
            hp, j = u
            psl = slice(64 * s, 64 * s + 64)
            qsl = slice(j * 512, (j + 1) * 512)
            st = state[u]
            avd, rrb = st["avd"][s], st["rrb"][s]
            rbc = bc_pool.tile([64, 512], bf16, tag="rbc", bufs=2)
            nc.gpsimd.partition_broadcast(rbc[:], rrb[:])
            nc.vector.tensor_mul(AO_sb[psl, hp, qsl], avd[:], rbc[:])
            yield None

        def gen_proj(t, alt=False):
            # partial output projection (pre-bias) for token tile t; the
            # two 512-wide halves each hold one psum buf so consecutive
            # halves pipeline. In the tail (alt=True) the second half
            # borrows the attn@V pool -- its accumulators are done -- for
            # twice the ring width.
            tsl = slice(t * 128, (t + 1) * 128)
            st = ostage.tile([128, D], bf16, tag="st")
            for half in range(2):
                osl = slice(512 * half, 512 * half + 512)
                if alt and half == 1:
                    ps = ps_av.tile([128, 512], f32, tag="av", name="pj_av")
                else:
                    ps = ps_misc.tile([128, 512], f32, tag="m", name="pj")
                for fc in range(4):
                    yield nc.tensor.matmul(ps[:], AO_sb[:, fc, tsl],
                                           wp_s[:, fc, osl],
                                           start=(fc == 0), stop=(fc == 3))
                # Psum drains can't go to GPSIMD (no PSUM access -- BIR
                # verifier rejects it), but the ACT engine is idle after
                # the last exp: in the tail it drains h0 via AF.Copy in
                # parallel with the DVE's h1.
                if alt:
                    if half == 0:
                        nc.scalar.activation(st[:, osl], ps[:], AF.Copy)
                    else:
                        nc.vector.tensor_copy(st[:, osl], ps[:])
                    if t == 15:
                        # last tile: ship halves separately so the final
                        # transfer is half-sized
                        nc.sync.dma_start(out=out[tsl, osl], in_=st[:, osl])
                else:
                    nc.vector.tensor_copy(st[:, osl], ps[:])
            if not (alt and t == 15):
                nc.sync.dma_start(out=out[tsl, :], in_=st[:])

        # Projections for blocks 0-2 are split in two phases: prework
        # accumulates fc 0-2 into psum and spills to an SBUF partial one
        # unit before the block's hp=3 norm lands (this feeds unit 12,
        # which otherwise starves -- its block has no ready proj or Q/K
        # prefetch work); the fc3 phase adds the last chunk's matmul to
        # the partial and ships the tile, with its DVE adds absorbed by
        # mid-unit slack. Block 3 ships v2-style whole in the tail.
        def gen_proj_pre(t):
            tsl = slice(t * 128, (t + 1) * 128)
            part = ostage.tile([128, D], bf16, tag="part", bufs=5)
            for half in range(2):
                osl = slice(512 * half, 512 * half + 512)
                ps = ps_misc.tile([128, 512], f32, tag="m", name="pjpre")
                for fc in range(3):
                    yield nc.tensor.matmul(ps[:], AO_sb[:, fc, tsl],
                                           wp_s[:, fc, osl],
                                           start=(fc == 0), stop=(fc == 2))
                nc.vector.tensor_copy(part[:, osl], ps[:])
            state["part", t] = part

        def gen_proj_fc3(t):
            # alternate psum pools per tile: 4 banks in rotation keep the
            # tail's fc3 burst matmul-paced instead of DVE-add-paced
            tsl = slice(t * 128, (t + 1) * 128)
            part = state["part", t]
            st = ostage.tile([128, D], bf16, tag="st")
            for half in range(2):
                osl = slice(512 * half, 512 * half + 512)
                ps = ps_misc.tile([128, 512], f32, tag="m", name="pjfc3")
                yield nc.tensor.matmul(ps[:], AO_sb[:, 3, tsl],
                                       wp_s[:, 3, osl],
                                       start=True, stop=True)
                # halves ship separately so each DMA pipelines behind its
                # own add instead of waiting for the whole tile
                nc.vector.tensor_add(st[:, osl], ps[:], part[:, osl])
                nc.sync.dma_start(out=out[tsl, osl], in_=st[:, osl])

        def pump(n, anchor=None):
            done = 0
            while done < n and bg:
                try:
                    mm = next(bg[0])
                except StopIteration:
                    bg.pop(0)
                    continue
                if mm is not None and anchor is not None:
                    add_dep_helper(mm.ins, anchor.ins, sync=False,
                                   reason="pump pacing")
                done += 1

        def emit_e_slot(u, kc):
            # energies for BOTH heads of the pair, one key chunk: two
            # 64x128 row tiles at (0,0)/(64,0) run concurrently on the
            # PE; one N=1024 exp covers both heads.
            hp, j = u
            qsl = slice(j * 512, (j + 1) * 512)
            ksl = slice(kc * 128, (kc + 1) * 128)
            pb = state[u]["pb"]
            e2 = ps_e.tile([128, 2, 512], f32, tag="e")
            hs = hp % 2
            mmA = nc.tensor.matmul(e2[:, 0, :], KT_sb[0:64, hs, ksl],
                                   QT_sb[0:64, hs, qsl], start=True, stop=True)
            mmB = nc.tensor.matmul(e2[:, 1, :], KT_sb[64:128, hs, ksl],
                                   QT_sb[64:128, hs, qsl], start=True, stop=True)
            add_dep_helper(mmB.ins, mmA.ins, sync=False, reason="pair glue")
            mmA.ins.bass_priority = eprio[0]
            mmB.ins.bass_priority = eprio[0] + 1
            eprio[0] += 2
            return nc.scalar.activation(pb[:, kc, :, :], e2[:], AF.Exp)

        def run_gen(g):
            for _ in g:
                pass

        # prologue: just K(0,0) then Q(0,0) -- the first energy slot needs
        # exactly these two; everything else (K(0,1..3), Q(0,1..3), bvb,
        # V tiles) is pumped between unit-0 energy slots in the order
        # their input DMAs land.
        run_gen(gen_qk_ntile(wk_s, bkT_s, KT_sb, 0, 0))
        run_gen(gen_qk_ntile(wq_s, bqT_s, QT_sb, 0, 0))

        # Per-unit pump budgets. Invariant: each unit's attn@V blocks and
        # norms MUST drain within the unit that queued them -- an av matmul
        # anchored on a LATER unit's act deadlocks against the pb_pool
        # rotation (next unit's exp needs the pb bank freed by those reads).
        PUMP = {0: 7, 1: 8, 12: 6, 13: 6, 14: 6, 15: 8}
        P = None   # previous pair-unit (attn@V source)
        for ui, u in enumerate(units):
            hp, j = u
            state[u] = {"pb": pb_pool.tile([128, 16, 2, 512], bf16,
                                           tag="pb", name="pb"),
                        "av": {}, "avd": {}, "rrb": {}}
            # heads 4-7 of V, emitted directly (not pumped/anchored:
            # pacing deps on late acts cycle the pool-alloc ordering).
            # Spread over units 4-9 (first consumer is unit 9's attn@V
            # pump), weighted toward the filler-starved units 5/8/9.
            V1 = {4: range(0, 3), 5: range(3, 7), 6: range(7, 9),
                  7: range(9, 11), 8: range(11, 13), 9: range(13, 16)}
            for t in V1.get(ui, ()):
                run_gen(gen_v_tile(t, 1))

            if ui == 0:
                # DMA-arrival-ordered fillers: Q(0,1) only needs the slabs
                # the prologue used; K(0,n)/Q(0,n) follow their xn slab,
                # V h0 tiles follow wv half 0.
                bg.append(gen_qk_ntile(wq_s, bqT_s, QT_sb, 0, 1))
                bg.append(gen_qk_ntile(wk_s, bkT_s, KT_sb, 0, 1))
                bg.append(gen_qk_ntile(wq_s, bqT_s, QT_sb, 0, 2))
                bg.append(gen_qk_ntile(wk_s, bkT_s, KT_sb, 0, 2))
                bg.append(gen_bvb())
                bg.extend(gen_v_tile(t, 0) for t in range(2))
                bg.append(gen_qk_ntile(wk_s, bkT_s, KT_sb, 0, 3))
                bg.extend(gen_v_tile(t, 0) for t in range(2, 4))
                bg.append(gen_qk_ntile(wq_s, bqT_s, QT_sb, 0, 3))
                bg.extend(gen_v_tile(t, 0) for t in range(4, 6))
            elif ui == 1:
                # V tiles 8-15 must be emitted before the attn@V matmuls
                # that consume them (unwritten regions carry no deps).
                bg.extend(gen_v_tile(t, 0) for t in range(6, 8))
                bg.append(gen_av_block(P, 0, 0, 8))
                bg.extend(gen_v_tile(t, 0) for t in range(8, 16))
                bg.append(gen_av_block(P, 0, 8, 16))
                bg.append(gen_av_block(P, 1, 0, 16))
                bg.append(gen_norm_b(P, 0))
                bg.append(gen_norm_b(P, 1))
            else:
                # attn@V chains of P, then P's normalization in the SAME
                # unit (norm is gpsimd+DVE only; its latency hides off
                # the PE/ACT path), so pair-3 projections land a unit
                # earlier and the tail shrinks.
                bg.append(gen_av_block(P, 0, 0, 16))
                bg.append(gen_av_block(P, 1, 0, 16))
                bg.append(gen_norm_b(P, 0))
                bg.append(gen_norm_b(P, 1))
                if j == 2 and hp < 3:
                    bg.extend(gen_qk_ntile(wk_s, bkT_s, KT_sb, hp + 1, n)
                              for n in range(4))
                elif j == 3 and hp < 3:
                    bg.extend(gen_qk_ntile(wq_s, bqT_s, QT_sb, hp + 1, n)
                              for n in range(4))
                # projection cascade for blocks 0-2: fc3 completes P's
                # query block (its hp=3 norm just ran), prework covers
                # this unit's block; block 3 ships whole in the tail
                if P[0] == 3 and P[1] < 3:
                    bg.extend(gen_proj_fc3(4 * P[1] + tt) for tt in range(4))
                if hp == 3 and j < 3:
                    bg.extend(gen_proj_pre(4 * j + tt) for tt in range(4))
            for kc in range(16):
                act = emit_e_slot(u, kc)
                pump(PUMP.get(ui, 4), act)
            P = u

        # pipeline tail: attn@V + normalization of (3,3) (each av matmul
        # only waits its own exp, so these chains run as the last slots'
        # exps land), then block 3's dense projections -- their fc 0-2
        # matmuls keep the PE fed while the final norm chain resolves,
        # and their copies overlap the PE work.
        bg.append(gen_av_block(P, 0, 0, 16))
        bg.append(gen_av_block(P, 1, 0, 16))
        bg.append(gen_norm_b(P, 0))
        bg.append(gen_norm_b(P, 1))
        bg.extend(gen_proj(12 + tt, alt=True) for tt in range(4))
        while bg:
            pump(1)


def get_program():
    if "nc" not in _prog_cache:
        _prog_cache["nc"] = _build_program()
    return _prog_cache["nc"]


def make_in_maps(inputs):
    x = np.asarray(inputs["x"], dtype=np.float32)
    Wq = np.asarray(inputs["Wq"], dtype=np.float32)
    bq = np.asarray(inputs["bq"], dtype=np.float32)
    Wk = np.asarray(inputs["Wk"], dtype=np.float32)
    bk = np.asarray(inputs["bk"], dtype=np.float32)
    Wv = np.asarray(inputs["Wv"], dtype=np.float32)
    bv = np.asarray(inputs["bv"], dtype=np.float32)
    Wp = np.asarray(inputs["Wp"], dtype=np.float32)

    def w_r(wT):
        # [1024(d_in), 512(f)] -> [p, hp, m, c] with d_in = m*128+p,
        # f = hp*128+c
        return np.ascontiguousarray(
            wT.reshape(8, 128, 4, 128).transpose(1, 2, 0, 3)).astype(BF16)

    ones_h = np.ones((1, 128), dtype=BF16)
    in_maps = []
    for c in range(N_CORES):
        b, half = divmod(c, 2)
        fs = slice(half * FH, half * FH + FH)
        xT = x[b].T                       # [1024(d), 2048(t)]
        xn_h = np.ascontiguousarray(
            xT.reshape(8, 128, 4, 512).transpose(1, 2, 0, 3)).astype(BF16)
        wvT = Wv[fs].T                    # [1024(d_in), 512(f)]
        wvr_h = np.ascontiguousarray(
            wvT.reshape(8, 128, 2, 256).transpose(1, 2, 0, 3)).astype(BF16)
        in_maps.append({
            "xn": xn_h,
            "wqr": w_r(Wq[fs].T),
            "wkr": w_r(Wk[fs].T),
            "wvr": wvr_h,
            "bqT": np.ascontiguousarray(bq[fs].reshape(4, 128).T),
            "bkT": np.ascontiguousarray(bk[fs].reshape(4, 128).T),
            "bvs": bv[fs].astype(BF16).reshape(1, FH),
            "wpT": np.ascontiguousarray(Wp[:, fs].T).astype(BF16),
            "ones": ones_h,
        })
    return in_maps


def gather_output(results, bp):
    bp = np.asarray(bp, dtype=np.float32)
    return np.stack([
        results[2 * b]["out"].astype(np.float32)
        + results[2 * b + 1]["out"].astype(np.float32) + bp[None, :]
        for b in range(4)
    ]).astype(np.float32)


def kernel(**inputs):
    nc = get_program()
    in_maps = make_in_maps(inputs)
    res = run_bass_kernel_spmd(nc, in_maps, list(range(N_CORES))).results
    return gather_output(res, inputs["bp"])
